# revision 4
# baseline (speedup 1.0000x reference)
"""Trainium2 Bass kernel for nn_AddPoolingFusion.

Reference computation (b=16, l1=l2=2048, d1=d2=d3=768):
    y1  = x1 @ W1.T + b1                      # [b, l1, d3]
    y2  = x2 @ W2.T + b2                      # [b, l2, d3]
    out = y1 + mean(y2, axis=1, keepdims=True)

Because the mean over l2 commutes with the linear layer:
    out[b,i,:] = x1[b,i] @ W1.T + c[b]
    c[b]       = (b1 + b2) + mean_j(x2[b,j]) @ W2.T

Strategy: data-parallel over batch, 2 batches per core, no collectives.
The per-core floor is the x1 matmul on TensorE: 32 m-tiles x 6 k-chunks
x (512+256) moving columns = 147456 PE cycles = 74.9us at the 2.0 GHz
sustained (P0) clock / 61.4us at 2.4 GHz. Schedule keeps that stream
dense from ~10us (end of the fixed ~7.4us framework preamble + first
DMA landing) to the end, with multi-us margins on every cross-engine
handoff:

- Rings: scalar carries all of W1 (done ~13.5us, then issues nothing),
  sync carries all of x1 (q0 split in halves so m-tile 0 starts ~9.9us)
  plus W2 mid-stream; x2 rides the GpSimd SWDGE ring, gated on x1 q0
  so it never steals HBM bandwidth from the critical path.
- Warm-up matmuls on junk data run during the initial DMA fill so the
  PE's HAM activity window flips to full clock early.
- x2 is pre-transposed on the host to [d2-partition, l2-free]; the
  per-batch mean is a free-dim reduction: 10 chunks on the Scalar
  engine (activation accum_out, idle after W1), the last 2 on Vector,
  pipelined with chunk arrival - all done ~35us. Engine-disjoint
  slices of the accumulator live in separate 16B words.
- c[b] = xbar2 @ W2.T for BOTH batches in one 12-matmul group (the two
  means are the 2 columns of the stationary operand), inserted after
  m-tile 11. Batch 0's row lands on partition 0 (partition_broadcast
  sources it directly, ready ~44us, first needed by t14's evac ~47.5);
  batch 1's goes through a tiny SBUF->SBUF hop on the gpsimd ring
  (ready ~47, first needed by t16's evac ~52).
- m-tiles 0-13 evacuate PSUM as plain copies (bias deferred until c is
  ready); t14+ evacuate with the bias add fused on the DVE. The 14
  deferred adds all run on the otherwise-idle GpSimd engine, so the
  DVE never head-blocks the PSUM evacuations; stores spread from
  ~50us on instead of bunching into the tail.
- Output stores: 786KB per 4-m-tile unit, alternating rings; the final
  unit splits 3+1 so the tail after the last matmul is one small DMA.

Host-side prep (layout/dtype only): partition-major SBUF images, bf16
casts (rel err ~3e-3 vs the 2e-2 gate), output stored bf16 and upcast
on the host. x2/W2 feed only the mean term (magnitude ~5% of output,
averaged over 2048 rows), so fp8 halves their HBM traffic at ~1e-3
output error.
"""

import sys

import numpy as np

# concourse normally comes from the axon site overlay already on sys.path;
# append /opt/trn_rl_repo as a fallback only.
if "/opt/trn_rl_repo" not in sys.path:
    sys.path.append("/opt/trn_rl_repo")

N_CORES = 8
B_PER_CORE = 2
L = 2048
D = 768  # d1 == d2 == d3 == 768
P = 128
NCH = D // P  # 6 contraction chunks
M = B_PER_CORE * L  # 4096 rows per core
NQ = 16  # x1 quarter groups, 2 m-tiles each
SPT = 4  # m-tiles per output store unit
NST = (M // P) // SPT  # 8 store units
K_DEFER = 14  # m-tiles evacuated before the bias c is ready
N_VRED = 2  # x2 chunks reduced on Vector (the rest on Scalar)


def build_nc(debug=False):
    import concourse.bacc as bacc
    import concourse.mybir as mybir
    import concourse.tile as tile

    f32 = mybir.dt.float32
    bf16 = mybir.dt.bfloat16
    fp8 = mybir.dt.float8e4
    add = mybir.AluOpType.add
    Copy = mybir.ActivationFunctionType.Copy
    AxX = mybir.AxisListType.X

    nc = bacc.Bacc(None, target_bir_lowering=False, debug=debug)

    x1h = nc.declare_dram_parameter("x1h", [NQ, P, NCH, 2 * P], bf16, isOutput=False)
    x2h = nc.declare_dram_parameter("x2h", [B_PER_CORE, P, NCH, L], fp8, isOutput=False)
    w1h = nc.declare_dram_parameter("w1h", [P, NCH, D], bf16, isOutput=False)
    w2h = nc.declare_dram_parameter("w2h", [P, NCH, D], fp8, isOutput=False)
    bsh = nc.declare_dram_parameter("bsh", [B_PER_CORE, D], f32, isOutput=False)
    outh = nc.declare_dram_parameter("outh", [NST, P, SPT, D], bf16, isOutput=True)

    with tile.TileContext(nc) as tc:
        with (
            tc.tile_pool(name="const", bufs=1) as const,
            tc.tile_pool(name="x1p", bufs=1) as x1p,
            tc.tile_pool(name="x2p", bufs=1) as x2p,
            tc.tile_pool(name="yp", bufs=1) as yp,
            tc.tile_pool(name="psY", bufs=3, space="PSUM") as psY,
            tc.tile_pool(name="psC", bufs=1, space="PSUM") as psC,
        ):
            warm = const.tile([P, 512], bf16)
            nc.vector.memset(warm[:], 0.03125)

            # ---- DMA kickoff ----
            # sync ring: x1 only (q0 in halves so m-tile 0 starts early),
            # with W2 slotted in mid-stream
            w1sb = const.tile([P, NCH, D], bf16)
            x1tiles = [
                x1p.tile([P, NCH, 2 * P], bf16, name=f"x1q{q}", tag=f"x1q{q}")
                for q in range(NQ)
            ]
            nc.sync.dma_start(x1tiles[0][:, :, 0:P], x1h[0][:, :, 0:P])
            q0bdma = nc.sync.dma_start(x1tiles[0][:, :, P : 2 * P], x1h[0][:, :, P : 2 * P])
            for q in (1, 2, 3, 4):
                nc.sync.dma_start(x1tiles[q][:], x1h[q])
            w2sb = const.tile([P, NCH, D], fp8)
            nc.sync.dma_start(w2sb[:], w2h[:])
            for q in range(5, NQ):
                nc.sync.dma_start(x1tiles[q][:], x1h[q])

            # scalar ring: all of W1 chunk by chunk, then the bias row;
            # after ~13.5us the scalar engine only does reduces
            for c in range(NCH):
                nc.scalar.dma_start(w1sb[:, c : c + 1, :], w1h[:, c : c + 1, :])
            bsum_sb = const.tile([B_PER_CORE, D], f32)
            nc.scalar.dma_start(bsum_sb[:], bsh[:])

            # x2 on the gpsimd SWDGE ring; chunk 0 yields HBM bandwidth
            # until x1 q0 has landed
            x2tiles = [
                x2p.tile([P, NCH, L], fp8, name=f"x2t{b}", tag=f"x2t{b}")
                for b in range(B_PER_CORE)
            ]
            for i in range(B_PER_CORE * NCH):
                b, c = i // NCH, i % NCH
                dma = nc.gpsimd.dma_start(x2tiles[b][:, c, :], x2h[b, :, c, :])
                if i == 0:
                    tile.add_dep_helper(
                        dma.ins, q0bdma.ins, sync=True, reason="x2 yields to x1 q0"
                    )

            # ---- warm-up matmuls: flip the HAM clock gate during DMA fill ----
            pc2 = psC.tile([B_PER_CORE, D], f32)
            for _ in range(4):
                nc.tensor.matmul(
                    pc2[0:1, 0:512], warm[:, 0:1], warm[:], start=True, stop=True
                )

            # ---- x2 mean accumulator ----
            # [P, b, chunk(+pad)]: scalar-written cells (i=0..9) and
            # vector-written cells (i=10,11) sit in different 16B words.
            xbtf = const.tile([P, B_PER_CORE, NCH + 2], f32)
            scr = const.tile([P, L], bf16)

            def emit_scalar_reduce(i):
                b, c = i // NCH, i % NCH
                nc.scalar.activation(
                    scr[:], x2tiles[b][:, c, :], Copy,
                    accum_out=xbtf[:, b, c : c + 1],
                )

            def emit_vector_reduce(i):
                b, c = i // NCH, i % NCH
                nc.vector.tensor_reduce(
                    xbtf[:, b, c : c + 1], x2tiles[b][:, c, :], AxX, add
                )

            for i in range(B_PER_CORE * NCH - N_VRED):
                emit_scalar_reduce(i)
            # the last N_VRED chunks reduce on the DVE, interleaved into
            # the m-tile loop (after evacs t5, t6) so they pipeline with
            # chunk arrival without head-blocking the evacuations
            vec_red_at = {5: 10, 6: 11}

            # ---- main matmul stream ----
            cr = [None, None]
            ysu = [
                yp.tile([P, SPT, D], bf16, name=f"ys{s}", tag=f"ys{s}")
                for s in range(NST)
            ]

            def emit_mtile(t):
                q, sub = t // 2, t % 2
                xq = x1tiles[q]
                py_ = psY.tile([P, D], f32)
                for c in range(NCH):
                    xw = xq[:, c, sub * P : (sub + 1) * P]
                    nc.tensor.matmul(
                        py_[:, 0:512], xw, w1sb[:, c, 0:512],
                        start=(c == 0), stop=(c == NCH - 1),
                    )
                    nc.tensor.matmul(
                        py_[:, 512:768], xw, w1sb[:, c, 512:768],
                        start=(c == 0), stop=(c == NCH - 1),
                    )
                s, tl = t // SPT, t % SPT
                if t < K_DEFER:
                    nc.vector.tensor_copy(ysu[s][:, tl, :], py_[:])
                else:
                    nc.vector.tensor_tensor(
                        ysu[s][:, tl, :], py_[:], cr[t // 16][:], op=add
                    )

            def emit_c_block():
                # c[0:2] = (mean(x2) @ W2.T + b1 + b2); both batches are the
                # 2 columns of the stationary operand -> 12 matmuls total
                xbtb = const.tile([P, B_PER_CORE, NCH], bf16)
                nc.gpsimd.tensor_scalar_mul(
                    xbtb[:], xbtf[:, :, 0:NCH], 1.0 / L
                )
                for c in range(NCH):
                    nc.tensor.matmul(
                        pc2[0:2, 0:512], xbtb[:, 0:2, c], w2sb[:, c, 0:512],
                        start=(c == 0), stop=(c == NCH - 1),
                    )
                for c in range(NCH):
                    nc.tensor.matmul(
                        pc2[0:2, 512:768], xbtb[:, 0:2, c], w2sb[:, c, 512:768],
                        start=(c == 0), stop=(c == NCH - 1),
                    )
                csb = const.tile([B_PER_CORE, D], bf16)
                nc.vector.tensor_tensor(
                    csb[:], pc2[0:2, :], bsum_sb[0:2, :], op=add
                )
                cr0 = const.tile([P, D], bf16, name="cr0", tag="cr0")
                nc.gpsimd.partition_broadcast(cr0[:], csb[0:1, :])
                # batch 1's row sits on partition 1: hop it to partition 0
                # via a tiny SBUF->SBUF DMA (idle gpsimd ring), then bcast.
                # Needed only by t16's evac, ~7us later.
                cs1 = const.tile([1, D], bf16)
                nc.gpsimd.dma_start(cs1[:], csb[1:2, :])
                cr1 = const.tile([P, D], bf16, name="cr1", tag="cr1")
                nc.gpsimd.partition_broadcast(cr1[:], cs1[:])
                cr[0], cr[1] = cr0, cr1

            def emit_store(s, ring, lo=0, hi=SPT):
                ring.dma_start(outh[s][:, lo:hi, :], ysu[s][:, lo:hi, :])

            store_ring = [nc.sync, nc.scalar]
            n_store = 0
            for t in range(32):
                if t == K_DEFER - 2:
                    emit_c_block()
                emit_mtile(t)
                if t in vec_red_at:
                    emit_vector_reduce(vec_red_at[t])
                if t >= K_DEFER and t % SPT == SPT - 1 and t < 31:
                    emit_store(t // SPT, store_ring[n_store % 2])
                    n_store += 1
                if t == 30:
                    emit_store(7, store_ring[n_store % 2], 0, 3)
                    n_store += 1
                if t == 31:
                    emit_store(7, store_ring[n_store % 2], 3, 4)
                    n_store += 1

            # deferred bias adds for t0..t13 (all batch 0) on the idle
            # GpSimd engine (the gpsimd queue reaches them right after the
            # c broadcast, ~46us); each store unit fires after its adds
            for s in range(K_DEFER // SPT + 1):
                hi = min(SPT, K_DEFER - s * SPT)
                for tl in range(hi):
                    nc.gpsimd.tensor_tensor(
                        ysu[s][:, tl, :], ysu[s][:, tl, :], cr[0][:], op=add
                    )
                if hi == SPT:
                    emit_store(s, store_ring[n_store % 2])
                    n_store += 1
                else:
                    # unit s3: tiles 12-13 deferred, 14-15 fused earlier
                    emit_store(s, store_ring[n_store % 2])
                    n_store += 1

    return nc


def make_in_maps(x1, x2, W1, b1, W2, b2):
    import ml_dtypes

    bf16 = ml_dtypes.bfloat16
    fp8 = ml_dtypes.float8_e4m3fn

    def wlayout(W, dt):
        # [e, d] -> W.T [d, e] -> [p, c, e] with d = c*128 + p
        wt = np.ascontiguousarray(W.T).reshape(NCH, P, D).transpose(1, 0, 2)
        return np.ascontiguousarray(wt).astype(dt)

    w1h = wlayout(W1, bf16)
    w2h = wlayout(W2, fp8)
    bsh = np.ascontiguousarray(
        np.broadcast_to((b1 + b2).reshape(1, D), (B_PER_CORE, D))
    ).astype(np.float32)
    in_maps = []
    for k in range(N_CORES):
        x1_s = x1[k * B_PER_CORE : (k + 1) * B_PER_CORE]  # [2, 2048, 768]
        x2_s = x2[k * B_PER_CORE : (k + 1) * B_PER_CORE]
        # x1t [d, m] with col m = b*2048 + i, then quarter-major image
        x1t = np.transpose(x1_s, (2, 0, 1)).reshape(D, M)
        x1h = np.ascontiguousarray(
            x1t.reshape(NCH, P, NQ, 2 * P).transpose(2, 1, 0, 3)
        ).astype(bf16)  # [q, p, c, m_in_quarter]
        # x2 transposed: [b, p, c, j] with d = c*128 + p
        x2h = np.ascontiguousarray(
            np.transpose(x2_s, (0, 2, 1)).reshape(B_PER_CORE, NCH, P, L)
            .transpose(0, 2, 1, 3)
        ).astype(fp8)
        in_maps.append(
            {"x1h": x1h, "x2h": x2h, "w1h": w1h, "w2h": w2h, "bsh": bsh}
        )
    return in_maps


def kernel(x1, x2, W1, b1, W2, b2, trace=False):
    from concourse.bass_utils import run_bass_kernel_spmd

    # accept jax arrays / lists transparently
    x1, x2, W1, b1, W2, b2 = (
        np.asarray(t, dtype=np.float32) for t in (x1, x2, W1, b1, W2, b2)
    )
    nc = build_nc(debug=False)
    nc.finalize()
    in_maps = make_in_maps(x1, x2, W1, b1, W2, b2)
    res = run_bass_kernel_spmd(
        nc, in_maps, core_ids=list(range(N_CORES)), trace=trace
    )
    shards = []
    for k in range(N_CORES):
        oh = res.results[k]["outh"]  # [NST, P, SPT, D] bf16, row = (s*SPT+t)*128+p
        flat = (
            oh.astype(np.float32).transpose(0, 2, 1, 3).reshape(M, D)
        )
        shards.append(flat.reshape(B_PER_CORE, L, D))
    out = np.concatenate(shards, axis=0)
    if trace:
        kernel.last_result = res
    return out


# revision 5
# speedup vs baseline: 1.0504x; 1.0504x over previous
"""Trainium2 Bass kernel for nn_AddPoolingFusion.

Reference computation (b=16, l1=l2=2048, d1=d2=d3=768):
    y1  = x1 @ W1.T + b1                      # [b, l1, d3]
    y2  = x2 @ W2.T + b2                      # [b, l2, d3]
    out = y1 + mean(y2, axis=1, keepdims=True)

Because the mean over l2 commutes with the linear layer:
    out[b,i,:] = x1[b,i] @ W1.T + c[b]
    c[b]       = (b1 + b2) + mean_j(x2[b,j]) @ W2.T

Strategy: data-parallel over batch, 2 batches per core, no collectives.
The per-core floor is the x1 matmul on TensorE: 32 m-tiles x 6 k-chunks
x (512+256) moving columns = 147456 PE cycles = 74.9us at the 2.0 GHz
sustained (P0) clock / 61.4us at 2.4 GHz (the clock depends on chip
power state run-to-run). Schedule keeps that stream dense from ~10us
(end of the fixed ~7.4us framework preamble + first DMA landing) to
the end, with multi-us margins on every cross-engine handoff:

- Rings: sync carries x1 (q0 split in halves so m-tile 0 starts ~10us)
  plus W1 odd chunks and W2; scalar carries W1 even chunks, bias, and
  8 of the 12 x2 chunks interleaved with their reductions; the other
  4 x2 chunks ride the GpSimd ring. All x2 DMAs are gated on x1 q0
  (sem dep on the scalar ring, data dep via a gate op on gpsimd) so
  they never steal HBM bandwidth from the critical path.
- Warm-up matmuls on junk data run during the initial DMA fill so the
  PE's HAM activity window flips to full clock early.
- x2 is pre-transposed on the host to [d2-partition, l2-free]; the
  per-batch mean is a free-dim reduction: 8 chunks on the Scalar
  engine (activation accum_out), 4 on Vector (slotted after evacs
  t4-t7), pipelined with chunk arrival - all done ~36us. The two
  engines accumulate into separate tiles (no shared-word writes).
- c[b] = xbar2 @ W2.T as two M=1 matmul passes sharing one PSUM tile
  (12 matmuls each), inserted after m-tile 12. Both results land on
  partition 0 where partition_broadcast sources them directly. Batch
  1 (needed first, by t16's fused evac) goes first.
- m-tiles 0-15 (batch 0) evacuate PSUM as plain copies; t16+ (batch 1)
  evacuate with the bias add fused on the DVE. The 16 deferred adds
  split: store units s0/s1 on the idle GpSimd engine, s2/s3 trickled
  1-per-slot on the DVE, so the DVE never head-blocks the PSUM
  evacuations; stores spread from ~50us on instead of bunching into
  the tail.
- Output stores: 786KB per 4-m-tile unit, alternating rings; the final
  unit splits 3+1 so the tail after the last matmul is one small DMA.

Host-side prep (layout/dtype only): partition-major SBUF images, bf16
casts (rel err ~3e-3 vs the 2e-2 gate), output stored bf16 and upcast
on the host. x2/W2 feed only the mean term (magnitude ~5% of output,
averaged over 2048 rows), so fp8 halves their HBM traffic at ~1e-3
output error.
"""

import sys

import numpy as np

# concourse normally comes from the axon site overlay already on sys.path;
# append /opt/trn_rl_repo as a fallback only.
if "/opt/trn_rl_repo" not in sys.path:
    sys.path.append("/opt/trn_rl_repo")

N_CORES = 8
B_PER_CORE = 2
L = 2048
D = 768  # d1 == d2 == d3 == 768
P = 128
NCH = D // P  # 6 contraction chunks
M = B_PER_CORE * L  # 4096 rows per core
NQ = 16  # x1 quarter groups, 2 m-tiles each
SPT = 4  # m-tiles per output store unit
NST = (M // P) // SPT  # 8 store units
K_DEFER = 16  # m-tiles evacuated before the bias c is ready (= batch 0)
N_VRED = 4  # x2 chunks reduced on Vector (the rest on Scalar)
C_AT = 13  # c-block emitted before this m-tile


def build_nc(debug=False):
    import concourse.bacc as bacc
    import concourse.mybir as mybir
    import concourse.tile as tile

    f32 = mybir.dt.float32
    bf16 = mybir.dt.bfloat16
    fp8 = mybir.dt.float8e4
    add = mybir.AluOpType.add
    Copy = mybir.ActivationFunctionType.Copy
    AxX = mybir.AxisListType.X

    nc = bacc.Bacc(None, target_bir_lowering=False, debug=debug)

    x1h = nc.declare_dram_parameter("x1h", [NQ, P, NCH, 2 * P], bf16, isOutput=False)
    x2h = nc.declare_dram_parameter("x2h", [B_PER_CORE, P, NCH, L], fp8, isOutput=False)
    w1h = nc.declare_dram_parameter("w1h", [P, NCH, D], bf16, isOutput=False)
    w2h = nc.declare_dram_parameter("w2h", [P, NCH, D], fp8, isOutput=False)
    bsh = nc.declare_dram_parameter("bsh", [B_PER_CORE, D], f32, isOutput=False)
    outh = nc.declare_dram_parameter("outh", [NST, P, SPT, D], bf16, isOutput=True)

    with tile.TileContext(nc) as tc:
        with (
            tc.tile_pool(name="const", bufs=1) as const,
            tc.tile_pool(name="x1p", bufs=1) as x1p,
            tc.tile_pool(name="x2p", bufs=1) as x2p,
            tc.tile_pool(name="yp", bufs=1) as yp,
            tc.tile_pool(name="psY", bufs=3, space="PSUM") as psY,
            tc.tile_pool(name="psC", bufs=1, space="PSUM") as psC,
        ):
            warm = const.tile([P, 512], bf16)
            nc.vector.memset(warm[:], 0.03125)

            # ---- DMA kickoff ----
            # sync ring: x1 q0 halves lead, then W1 odd chunks, rest of x1,
            # W2 slotted in mid-stream
            w1sb = const.tile([P, NCH, D], bf16)
            x1tiles = [
                x1p.tile([P, NCH, 2 * P], bf16, name=f"x1q{q}", tag=f"x1q{q}")
                for q in range(NQ)
            ]
            nc.sync.dma_start(x1tiles[0][:, :, 0:P], x1h[0][:, :, 0:P])
            q0bdma = nc.sync.dma_start(
                x1tiles[0][:, :, P : 2 * P], x1h[0][:, :, P : 2 * P]
            )
            for c in (1, 3, 5):
                nc.sync.dma_start(w1sb[:, c : c + 1, :], w1h[:, c : c + 1, :])
            for q in (1, 2, 3, 4):
                nc.sync.dma_start(x1tiles[q][:], x1h[q])
            w2sb = const.tile([P, NCH, D], fp8)
            nc.sync.dma_start(w2sb[:], w2h[:])
            for q in range(5, NQ):
                nc.sync.dma_start(x1tiles[q][:], x1h[q])

            # scalar ring: W1 even chunks + bias row
            for c in (0, 2, 4):
                nc.scalar.dma_start(w1sb[:, c : c + 1, :], w1h[:, c : c + 1, :])
            bsum_sb = const.tile([B_PER_CORE, D], f32)
            nc.scalar.dma_start(bsum_sb[:], bsh[:])

            x2tiles = [
                x2p.tile([P, NCH, L], fp8, name=f"x2t{b}", tag=f"x2t{b}")
                for b in range(B_PER_CORE)
            ]
            # the 4 Vector-reduced chunks (b1 c2..c5) ride the gpsimd ring,
            # gated on x1 q0 via a data dep (a tiny gpsimd op that reads q0)
            gate = const.tile([1, 4], bf16)
            nc.gpsimd.tensor_scalar_mul(gate[:], x1tiles[0][0:1, 0, P : P + 4], 1.0)
            for c in range(2, NCH):
                nc.gpsimd.dma_start(x2tiles[1][:, c, :], x2h[1, :, c, :])

            # ---- warm-up matmuls: flip the HAM clock gate during DMA fill ----
            pc = psC.tile([1, D], f32)
            for _ in range(4):
                nc.tensor.matmul(
                    pc[:, 0:512], warm[:, 0:1], warm[:], start=True, stop=True
                )

            # ---- x2 mean accumulators (separate tiles per engine) ----
            xbts = const.tile([P, B_PER_CORE, NCH], f32)  # scalar-written
            xbtv = const.tile([P, N_VRED], f32)  # vector-written
            scr = const.tile([P, L], bf16)

            # scalar ring: 8 x2 chunks (b0 c0..c5, b1 c0..c1) interleaved
            # with their reductions, 3 transfers ahead; first chunk gated
            # on x1 q0
            s_chunks = [(0, c) for c in range(NCH)] + [(1, 0), (1, 1)]

            def x2_sdma(j):
                b, c = s_chunks[j]
                dma = nc.scalar.dma_start(x2tiles[b][:, c, :], x2h[b, :, c, :])
                if j == 0:
                    tile.add_dep_helper(
                        dma.ins, q0bdma.ins, sync=True, reason="x2 yields to x1 q0"
                    )

            for j in range(3):
                x2_sdma(j)
            for j in range(len(s_chunks)):
                b, c = s_chunks[j]
                nc.scalar.activation(
                    scr[:], x2tiles[b][:, c, :], Copy,
                    accum_out=xbts[:, b, c : c + 1],
                )
                if j + 3 < len(s_chunks):
                    x2_sdma(j + 3)

            def emit_vector_reduce(v):
                nc.vector.tensor_reduce(
                    xbtv[:, v : v + 1], x2tiles[1][:, 2 + v, :], AxX, add
                )

            vec_red_at = {4: 0, 5: 1, 6: 2, 7: 3}

            # ---- main matmul stream ----
            cr = [None, None]
            ysu = [
                yp.tile([P, SPT, D], bf16, name=f"ys{s}", tag=f"ys{s}")
                for s in range(NST)
            ]

            def emit_mtile(t):
                q, sub = t // 2, t % 2
                xq = x1tiles[q]
                py_ = psY.tile([P, D], f32)
                for c in range(NCH):
                    xw = xq[:, c, sub * P : (sub + 1) * P]
                    nc.tensor.matmul(
                        py_[:, 0:512], xw, w1sb[:, c, 0:512],
                        start=(c == 0), stop=(c == NCH - 1),
                    )
                    nc.tensor.matmul(
                        py_[:, 512:768], xw, w1sb[:, c, 512:768],
                        start=(c == 0), stop=(c == NCH - 1),
                    )
                s, tl = t // SPT, t % SPT
                if t < K_DEFER:
                    nc.vector.tensor_copy(ysu[s][:, tl, :], py_[:])
                else:
                    nc.vector.tensor_tensor(
                        ysu[s][:, tl, :], py_[:], cr[t // 16][:], op=add
                    )

            def emit_c_block():
                # c[b] = (mean(x2[b]) @ W2.T + b1 + b2), two M=1 passes
                # sharing one PSUM tile so both rows land on partition 0.
                # Batch 1 first: its broadcast feeds t16's fused evac.
                xbtb = const.tile([P, B_PER_CORE, NCH], bf16)
                nc.gpsimd.tensor_scalar_mul(
                    xbtb[:, 0:2, 0:2], xbts[:, 0:2, 0:2], 1.0 / L
                )
                nc.gpsimd.tensor_scalar_mul(
                    xbtb[:, 0, 2:NCH], xbts[:, 0, 2:NCH], 1.0 / L
                )
                nc.gpsimd.tensor_scalar_mul(
                    xbtb[:, 1, 2:NCH], xbtv[:], 1.0 / L
                )
                for b in (1, 0):
                    for c in range(NCH):
                        nc.tensor.matmul(
                            pc[:, 0:512], xbtb[:, b, c : c + 1], w2sb[:, c, 0:512],
                            start=(c == 0), stop=(c == NCH - 1),
                        )
                    for c in range(NCH):
                        nc.tensor.matmul(
                            pc[:, 512:768], xbtb[:, b, c : c + 1],
                            w2sb[:, c, 512:768],
                            start=(c == 0), stop=(c == NCH - 1),
                        )
                    csb = const.tile([1, D], bf16, name=f"cs{b}", tag=f"cs{b}")
                    nc.vector.tensor_tensor(
                        csb[:], pc[:], bsum_sb[0:1, :], op=add
                    )
                    crb = const.tile([P, D], bf16, name=f"cr{b}", tag=f"cr{b}")
                    nc.gpsimd.partition_broadcast(crb[:], csb[:])
                    cr[b] = crb

            def emit_store(s, ring, lo=0, hi=SPT):
                ring.dma_start(outh[s][:, lo:hi, :], ysu[s][:, lo:hi, :])

            store_ring = [nc.sync, nc.scalar]
            n_store = 0
            dve_flush = [(s, tl) for s in (2, 3) for tl in range(SPT)]
            fi = 0
            for t in range(32):
                if t == C_AT:
                    emit_c_block()
                emit_mtile(t)
                if t in vec_red_at:
                    emit_vector_reduce(vec_red_at[t])
                if t == C_AT + 1:
                    # deferred adds for units s0/s1 on the idle GpSimd
                    # engine (queue reaches them right after cr0's bcast)
                    for s in (0, 1):
                        for tl in range(SPT):
                            nc.gpsimd.tensor_tensor(
                                ysu[s][:, tl, :], ysu[s][:, tl, :], cr[0][:],
                                op=add,
                            )
                        emit_store(s, store_ring[n_store % 2])
                        n_store += 1
                if t >= C_AT + 4 and fi < len(dve_flush):
                    # units s2/s3 trickle 1-per-slot on the DVE
                    s, tl = dve_flush[fi]
                    nc.vector.tensor_tensor(
                        ysu[s][:, tl, :], ysu[s][:, tl, :], cr[0][:], op=add
                    )
                    fi += 1
                    if tl == SPT - 1:
                        emit_store(s, store_ring[n_store % 2])
                        n_store += 1
                if t >= K_DEFER and t % SPT == SPT - 1 and t < 31:
                    emit_store(t // SPT, store_ring[n_store % 2])
                    n_store += 1
                if t == 30:
                    emit_store(7, store_ring[n_store % 2], 0, 3)
                    n_store += 1
                if t == 31:
                    emit_store(7, store_ring[n_store % 2], 3, 4)
                    n_store += 1
            while fi < len(dve_flush):
                s, tl = dve_flush[fi]
                nc.vector.tensor_tensor(
                    ysu[s][:, tl, :], ysu[s][:, tl, :], cr[0][:], op=add
                )
                fi += 1
                if tl == SPT - 1:
                    emit_store(s, store_ring[n_store % 2])
                    n_store += 1

    return nc


def make_in_maps(x1, x2, W1, b1, W2, b2):
    import ml_dtypes

    bf16 = ml_dtypes.bfloat16
    fp8 = ml_dtypes.float8_e4m3fn

    def wlayout(W, dt):
        # [e, d] -> W.T [d, e] -> [p, c, e] with d = c*128 + p
        wt = np.ascontiguousarray(W.T).reshape(NCH, P, D).transpose(1, 0, 2)
        return np.ascontiguousarray(wt).astype(dt)

    w1h = wlayout(W1, bf16)
    w2h = wlayout(W2, fp8)
    bsh = np.ascontiguousarray(
        np.broadcast_to((b1 + b2).reshape(1, D), (B_PER_CORE, D))
    ).astype(np.float32)
    in_maps = []
    for k in range(N_CORES):
        x1_s = x1[k * B_PER_CORE : (k + 1) * B_PER_CORE]  # [2, 2048, 768]
        x2_s = x2[k * B_PER_CORE : (k + 1) * B_PER_CORE]
        # x1t [d, m] with col m = b*2048 + i, then quarter-major image
        x1t = np.transpose(x1_s, (2, 0, 1)).reshape(D, M)
        x1h = np.ascontiguousarray(
            x1t.reshape(NCH, P, NQ, 2 * P).transpose(2, 1, 0, 3)
        ).astype(bf16)  # [q, p, c, m_in_quarter]
        # x2 transposed: [b, p, c, j] with d = c*128 + p
        x2h = np.ascontiguousarray(
            np.transpose(x2_s, (0, 2, 1)).reshape(B_PER_CORE, NCH, P, L)
            .transpose(0, 2, 1, 3)
        ).astype(fp8)
        in_maps.append(
            {"x1h": x1h, "x2h": x2h, "w1h": w1h, "w2h": w2h, "bsh": bsh}
        )
    return in_maps


def kernel(x1, x2, W1, b1, W2, b2, trace=False):
    from concourse.bass_utils import run_bass_kernel_spmd

    # accept jax arrays / lists transparently
    x1, x2, W1, b1, W2, b2 = (
        np.asarray(t, dtype=np.float32) for t in (x1, x2, W1, b1, W2, b2)
    )
    nc = build_nc(debug=False)
    nc.finalize()
    in_maps = make_in_maps(x1, x2, W1, b1, W2, b2)
    res = run_bass_kernel_spmd(
        nc, in_maps, core_ids=list(range(N_CORES)), trace=trace
    )
    shards = []
    for k in range(N_CORES):
        oh = res.results[k]["outh"]  # [NST, P, SPT, D] bf16, row = (s*SPT+t)*128+p
        flat = (
            oh.astype(np.float32).transpose(0, 2, 1, 3).reshape(M, D)
        )
        shards.append(flat.reshape(B_PER_CORE, L, D))
    out = np.concatenate(shards, axis=0)
    if trace:
        kernel.last_result = res
    return out


# revision 6
# speedup vs baseline: 1.2773x; 1.2160x over previous
"""Trainium2 Bass kernel for nn_AddPoolingFusion.

Reference computation (b=16, l1=l2=2048, d1=d2=d3=768):
    y1  = x1 @ W1.T + b1                      # [b, l1, d3]
    y2  = x2 @ W2.T + b2                      # [b, l2, d3]
    out = y1 + mean(y2, axis=1, keepdims=True)

Because the mean over l2 commutes with the linear layer:
    out[b,i,:] = x1[b,i] @ W1.T + c[b]
    c[b]       = (b1 + b2) + mean_j(x2[b,j]) @ W2.T

Strategy: data-parallel over batch, 2 batches per core, no collectives.
The per-core floor is the x1 matmul on TensorE: 32 m-tiles x 6 k-chunks
x (512+256) moving columns = 147456 PE cycles = 74.9us at the 2.0 GHz
sustained (P0) clock / 61.4us at 2.4 GHz (the clock depends on chip
power state run-to-run). Schedule keeps that stream dense from ~10us
(end of the fixed ~7.4us framework preamble + first DMA landing) to
the end, with multi-us margins on every cross-engine handoff:

- Rings: sync carries x1 (q0 split in halves so m-tile 0 starts ~10us)
  plus W1 odd chunks and W2; scalar carries W1 even chunks, bias, and
  8 of the 12 x2 chunks interleaved with their reductions; the other
  4 x2 chunks ride the GpSimd ring. All x2 DMAs are gated on x1 q0
  (sem dep on the scalar ring, data dep via a gate op on gpsimd) so
  they never steal HBM bandwidth from the critical path.
- Warm-up matmuls on junk data run during the initial DMA fill so the
  PE's HAM activity window flips to full clock early.
- x2 is pre-transposed on the host to [d2-partition, l2-free]; the
  per-batch mean is a free-dim reduction: 8 chunks on the Scalar
  engine (activation accum_out), 4 on Vector (slotted after evacs
  t4-t7), pipelined with chunk arrival - all done ~36us. The two
  engines accumulate into separate tiles (no shared-word writes).
- c[b] = xbar2 @ W2.T as two M=1 matmul passes sharing one PSUM tile
  (12 matmuls each), inserted after m-tile 12. Both results land on
  partition 0 where partition_broadcast sources them directly. Batch
  1 (needed first, by t16's fused evac) goes first.
- m-tiles 0-15 (batch 0) evacuate PSUM as plain copies; t16+ (batch 1)
  evacuate with the bias add fused on the DVE. The 16 deferred adds
  split: store units s0/s1 on the idle GpSimd engine, s2/s3 trickled
  1-per-slot on the DVE, so the DVE never head-blocks the PSUM
  evacuations; stores spread from ~50us on instead of bunching into
  the tail.
- Output stores: 786KB per 4-m-tile unit, alternating rings; the final
  unit splits 3+1 so the tail after the last matmul is one small DMA.

Host-side prep (layout/dtype only): partition-major SBUF images, bf16
casts (rel err ~3e-3 vs the 2e-2 gate), output stored bf16 and upcast
on the host. x2/W2 feed only the mean term (magnitude ~5% of output,
averaged over 2048 rows), so fp8 halves their HBM traffic at ~1e-3
output error.
"""

import sys

import numpy as np

# concourse normally comes from the axon site overlay already on sys.path;
# append /opt/trn_rl_repo as a fallback only.
if "/opt/trn_rl_repo" not in sys.path:
    sys.path.append("/opt/trn_rl_repo")

N_CORES = 8
B_PER_CORE = 2
L = 2048
D = 768  # d1 == d2 == d3 == 768
P = 128
NCH = D // P  # 6 contraction chunks
M = B_PER_CORE * L  # 4096 rows per core
NQ = 16  # x1 quarter groups, 2 m-tiles each
SPT = 4  # m-tiles per output store unit
NST = (M // P) // SPT  # 8 store units
K_DEFER = 16  # m-tiles evacuated before the bias c is ready (= batch 0)
N_VRED = 5  # x2 chunks reduced on Vector (the rest on Scalar)
C_AT = 16  # c-block emitted before this m-tile


def build_nc(debug=False):
    import concourse.bacc as bacc
    import concourse.mybir as mybir
    import concourse.tile as tile

    f32 = mybir.dt.float32
    bf16 = mybir.dt.bfloat16
    fp8 = mybir.dt.float8e4
    add = mybir.AluOpType.add
    Copy = mybir.ActivationFunctionType.Copy
    AxX = mybir.AxisListType.X

    nc = bacc.Bacc(None, target_bir_lowering=False, debug=debug)

    x1h = nc.declare_dram_parameter("x1h", [NQ, P, NCH, 2 * P], bf16, isOutput=False)
    x2h = nc.declare_dram_parameter("x2h", [B_PER_CORE, P, NCH, L], fp8, isOutput=False)
    w1h = nc.declare_dram_parameter("w1h", [P, NCH, D], bf16, isOutput=False)
    w2h = nc.declare_dram_parameter("w2h", [P, NCH, D], fp8, isOutput=False)
    bsh = nc.declare_dram_parameter("bsh", [B_PER_CORE, D], f32, isOutput=False)
    outh = nc.declare_dram_parameter("outh", [NST, P, SPT, D], bf16, isOutput=True)

    with tile.TileContext(nc) as tc:
        with (
            tc.tile_pool(name="const", bufs=1) as const,
            tc.tile_pool(name="x1p", bufs=1) as x1p,
            tc.tile_pool(name="x2p", bufs=1) as x2p,
            tc.tile_pool(name="yp", bufs=1) as yp,
            tc.tile_pool(name="psY", bufs=3, space="PSUM") as psY,
            tc.tile_pool(name="psC", bufs=1, space="PSUM") as psC,
        ):
            warm = const.tile([P, 512], bf16)
            nc.vector.memset(warm[:], 0.03125)

            # ---- DMA kickoff ----
            # sync ring: x1 q0 halves lead, then W1 odd chunks, rest of x1,
            # W2 slotted in mid-stream
            w1sb = const.tile([P, NCH, D], bf16)
            x1tiles = [
                x1p.tile([P, NCH, 2 * P], bf16, name=f"x1q{q}", tag=f"x1q{q}")
                for q in range(NQ)
            ]
            q0adma = nc.sync.dma_start(x1tiles[0][:, :, 0:P], x1h[0][:, :, 0:P])
            nc.sync.dma_start(
                x1tiles[0][:, :, P : 2 * P], x1h[0][:, :, P : 2 * P]
            )
            for c in (1, 3, 5):
                nc.sync.dma_start(w1sb[:, c : c + 1, :], w1h[:, c : c + 1, :])
            for q in (1, 2, 3, 4):
                nc.sync.dma_start(x1tiles[q][:], x1h[q])
            w2sb = const.tile([P, NCH, D], fp8)
            nc.sync.dma_start(w2sb[:], w2h[:])
            for q in range(5, NQ):
                nc.sync.dma_start(x1tiles[q][:], x1h[q])

            # scalar ring: W1 even chunks + bias row
            for c in (0, 2, 4):
                nc.scalar.dma_start(w1sb[:, c : c + 1, :], w1h[:, c : c + 1, :])
            bsum_sb = const.tile([B_PER_CORE, D], f32)
            nc.scalar.dma_start(bsum_sb[:], bsh[:])

            x2tiles = [
                x2p.tile([P, NCH, L], fp8, name=f"x2t{b}", tag=f"x2t{b}")
                for b in range(B_PER_CORE)
            ]

            # ---- warm-up matmuls: flip the HAM clock gate during DMA fill ----
            pc = psC.tile([1, D], f32)
            for _ in range(4):
                nc.tensor.matmul(
                    pc[:, 0:512], warm[:, 0:1], warm[:], start=True, stop=True
                )

            # ---- x2 mean accumulators (separate tiles per engine) ----
            xbts = const.tile([P, B_PER_CORE, NCH], f32)  # scalar-written
            xbtv = const.tile([P, N_VRED], f32)  # vector-written
            scr = const.tile([P, L], bf16)

            # all 12 x2 chunks ride the scalar ring, gated on x1 q0a via
            # the ring's FIFO (first DMA carries the sem dep). Chunk order:
            # b1 c0..c4 (Vector-reduced, needed first), then b1 c5 + b0
            # c0..c5 (Scalar-reduced) interleaved with their reductions so
            # the reduce chain paces the later transfers.
            v_chunks = [(1, c) for c in range(N_VRED)]
            s_chunks = [(1, c) for c in range(N_VRED, NCH)] + [
                (0, c) for c in range(NCH)
            ]

            def x2_sdma(b, c, gate=False):
                dma = nc.scalar.dma_start(x2tiles[b][:, c, :], x2h[b, :, c, :])
                if gate:
                    tile.add_dep_helper(
                        dma.ins, q0adma.ins, sync=True, reason="x2 yields to x1 q0"
                    )

            for j, (b, c) in enumerate(v_chunks):
                x2_sdma(b, c, gate=(j == 0))
            for j in range(3):
                x2_sdma(*s_chunks[j])
            for j, (b, c) in enumerate(s_chunks):
                nc.scalar.activation(
                    scr[:], x2tiles[b][:, c, :], Copy,
                    accum_out=xbts[:, b, c : c + 1],
                )
                if j + 3 < len(s_chunks):
                    x2_sdma(*s_chunks[j + 3])

            def emit_vector_reduce(v):
                nc.vector.tensor_reduce(
                    xbtv[:, v : v + 1], x2tiles[1][:, v, :], AxX, add
                )

            vec_red_at = {2: 0, 4: 1, 6: 2, 8: 3, 10: 4}

            # ---- main matmul stream ----
            cr = [None, None]
            ysu = [
                yp.tile([P, SPT, D], bf16, name=f"ys{s}", tag=f"ys{s}")
                for s in range(NST)
            ]

            def emit_mtile(t):
                q, sub = t // 2, t % 2
                xq = x1tiles[q]
                py_ = psY.tile([P, D], f32)
                for c in range(NCH):
                    xw = xq[:, c, sub * P : (sub + 1) * P]
                    nc.tensor.matmul(
                        py_[:, 0:512], xw, w1sb[:, c, 0:512],
                        start=(c == 0), stop=(c == NCH - 1),
                    )
                    nc.tensor.matmul(
                        py_[:, 512:768], xw, w1sb[:, c, 512:768],
                        start=(c == 0), stop=(c == NCH - 1),
                    )
                s, tl = t // SPT, t % SPT
                if t < K_DEFER:
                    nc.vector.tensor_copy(ysu[s][:, tl, :], py_[:])
                else:
                    nc.vector.tensor_tensor(
                        ysu[s][:, tl, :], py_[:], cr[t // 16][:], op=add
                    )

            def emit_c_block():
                # c[b] = (mean(x2[b]) @ W2.T + b1 + b2), two M=1 passes
                # sharing one PSUM tile so both rows land on partition 0.
                # Batch 1 first: its broadcast feeds t16's fused evac.
                xbtb = const.tile([P, B_PER_CORE, NCH], bf16)
                nc.gpsimd.tensor_scalar_mul(
                    xbtb[:, 0, :], xbts[:, 0, :], 1.0 / L
                )
                nc.gpsimd.tensor_scalar_mul(
                    xbtb[:, 1, N_VRED:NCH], xbts[:, 1, N_VRED:NCH], 1.0 / L
                )
                nc.gpsimd.tensor_scalar_mul(
                    xbtb[:, 1, 0:N_VRED], xbtv[:], 1.0 / L
                )
                for b in (1, 0):
                    for c in range(NCH):
                        nc.tensor.matmul(
                            pc[:, 0:512], xbtb[:, b, c : c + 1], w2sb[:, c, 0:512],
                            start=(c == 0), stop=(c == NCH - 1),
                        )
                    for c in range(NCH):
                        nc.tensor.matmul(
                            pc[:, 512:768], xbtb[:, b, c : c + 1],
                            w2sb[:, c, 512:768],
                            start=(c == 0), stop=(c == NCH - 1),
                        )
                    csb = const.tile([1, D], bf16, name=f"cs{b}", tag=f"cs{b}")
                    nc.vector.tensor_tensor(
                        csb[:], pc[:], bsum_sb[0:1, :], op=add
                    )
                    crb = const.tile([P, D], bf16, name=f"cr{b}", tag=f"cr{b}")
                    nc.gpsimd.partition_broadcast(crb[:], csb[:])
                    cr[b] = crb

            def emit_store(s, ring, lo=0, hi=SPT):
                ring.dma_start(outh[s][:, lo:hi, :], ysu[s][:, lo:hi, :])

            store_ring = [nc.sync, nc.scalar]
            n_store = 0
            dve_flush = [(s, tl) for s in (2, 3) for tl in range(SPT)]
            fi = 0
            for t in range(32):
                if t == C_AT:
                    emit_c_block()
                emit_mtile(t)
                if t in vec_red_at:
                    emit_vector_reduce(vec_red_at[t])
                if t == C_AT + 1:
                    # deferred adds for units s0/s1 on the idle GpSimd
                    # engine (queue reaches them right after cr0's bcast)
                    for s in (0, 1):
                        for tl in range(SPT):
                            nc.gpsimd.tensor_tensor(
                                ysu[s][:, tl, :], ysu[s][:, tl, :], cr[0][:],
                                op=add,
                            )
                        emit_store(s, store_ring[n_store % 2])
                        n_store += 1
                if t >= C_AT + 4 and fi < len(dve_flush):
                    # units s2/s3 trickle 1-per-slot on the DVE
                    s, tl = dve_flush[fi]
                    nc.vector.tensor_tensor(
                        ysu[s][:, tl, :], ysu[s][:, tl, :], cr[0][:], op=add
                    )
                    fi += 1
                    if tl == SPT - 1:
                        emit_store(s, store_ring[n_store % 2])
                        n_store += 1
                if t >= K_DEFER and t % SPT == SPT - 1 and t < 31:
                    emit_store(t // SPT, store_ring[n_store % 2])
                    n_store += 1
                if t == 30:
                    emit_store(7, store_ring[n_store % 2], 0, 3)
                    n_store += 1
                if t == 31:
                    emit_store(7, store_ring[n_store % 2], 3, 4)
                    n_store += 1
            while fi < len(dve_flush):
                s, tl = dve_flush[fi]
                nc.vector.tensor_tensor(
                    ysu[s][:, tl, :], ysu[s][:, tl, :], cr[0][:], op=add
                )
                fi += 1
                if tl == SPT - 1:
                    emit_store(s, store_ring[n_store % 2])
                    n_store += 1

    return nc


def make_in_maps(x1, x2, W1, b1, W2, b2):
    import ml_dtypes

    bf16 = ml_dtypes.bfloat16
    fp8 = ml_dtypes.float8_e4m3fn

    def wlayout(W, dt):
        # [e, d] -> W.T [d, e] -> [p, c, e] with d = c*128 + p
        wt = np.ascontiguousarray(W.T).reshape(NCH, P, D).transpose(1, 0, 2)
        return np.ascontiguousarray(wt).astype(dt)

    w1h = wlayout(W1, bf16)
    w2h = wlayout(W2, fp8)
    bsh = np.ascontiguousarray(
        np.broadcast_to((b1 + b2).reshape(1, D), (B_PER_CORE, D))
    ).astype(np.float32)
    in_maps = []
    for k in range(N_CORES):
        x1_s = x1[k * B_PER_CORE : (k + 1) * B_PER_CORE]  # [2, 2048, 768]
        x2_s = x2[k * B_PER_CORE : (k + 1) * B_PER_CORE]
        # x1t [d, m] with col m = b*2048 + i, then quarter-major image
        x1t = np.transpose(x1_s, (2, 0, 1)).reshape(D, M)
        x1h = np.ascontiguousarray(
            x1t.reshape(NCH, P, NQ, 2 * P).transpose(2, 1, 0, 3)
        ).astype(bf16)  # [q, p, c, m_in_quarter]
        # x2 transposed: [b, p, c, j] with d = c*128 + p
        x2h = np.ascontiguousarray(
            np.transpose(x2_s, (0, 2, 1)).reshape(B_PER_CORE, NCH, P, L)
            .transpose(0, 2, 1, 3)
        ).astype(fp8)
        in_maps.append(
            {"x1h": x1h, "x2h": x2h, "w1h": w1h, "w2h": w2h, "bsh": bsh}
        )
    return in_maps


def kernel(x1, x2, W1, b1, W2, b2, trace=False):
    from concourse.bass_utils import run_bass_kernel_spmd

    # accept jax arrays / lists transparently
    x1, x2, W1, b1, W2, b2 = (
        np.asarray(t, dtype=np.float32) for t in (x1, x2, W1, b1, W2, b2)
    )
    nc = build_nc(debug=False)
    nc.finalize()
    in_maps = make_in_maps(x1, x2, W1, b1, W2, b2)
    res = run_bass_kernel_spmd(
        nc, in_maps, core_ids=list(range(N_CORES)), trace=trace
    )
    shards = []
    for k in range(N_CORES):
        oh = res.results[k]["outh"]  # [NST, P, SPT, D] bf16, row = (s*SPT+t)*128+p
        flat = (
            oh.astype(np.float32).transpose(0, 2, 1, 3).reshape(M, D)
        )
        shards.append(flat.reshape(B_PER_CORE, L, D))
    out = np.concatenate(shards, axis=0)
    if trace:
        kernel.last_result = res
    return out


# revision 8
# speedup vs baseline: 1.2817x; 1.0035x over previous
"""Trainium2 Bass kernel for nn_AddPoolingFusion.

Reference computation (b=16, l1=l2=2048, d1=d2=d3=768):
    y1  = x1 @ W1.T + b1                      # [b, l1, d3]
    y2  = x2 @ W2.T + b2                      # [b, l2, d3]
    out = y1 + mean(y2, axis=1, keepdims=True)

Because the mean over l2 commutes with the linear layer:
    out[b,i,:] = x1[b,i] @ W1.T + c[b]
    c[b]       = (b1 + b2) + mean_j(x2[b,j]) @ W2.T

Strategy: data-parallel over batch, 2 batches per core, no collectives.
The per-core floor is the x1 matmul on TensorE: 32 m-tiles x 6 k-chunks
x (512+256) moving columns = 147456 PE cycles = 74.9us at the 2.0 GHz
sustained (P0) clock / 61.4us at 2.4 GHz (the clock depends on chip
power state run-to-run). Schedule keeps that stream dense from ~10us
(end of the fixed ~7.4us framework preamble + first DMA landing) to
the end, with multi-us margins on every cross-engine handoff:

- Rings: sync carries x1 (q0 split in halves so m-tile 0 starts ~10us)
  plus W1 odd chunks and W2; scalar carries W1 even chunks, bias, and
  8 of the 12 x2 chunks interleaved with their reductions; the other
  4 x2 chunks ride the GpSimd ring. All x2 DMAs are gated on x1 q0
  (sem dep on the scalar ring, data dep via a gate op on gpsimd) so
  they never steal HBM bandwidth from the critical path.
- Warm-up matmuls on junk data run during the initial DMA fill so the
  PE's HAM activity window flips to full clock early.
- x2 is pre-transposed on the host to [d2-partition, l2-free]; the
  per-batch mean is a free-dim reduction: 8 chunks on the Scalar
  engine (activation accum_out), 4 on Vector (slotted after evacs
  t4-t7), pipelined with chunk arrival - all done ~36us. The two
  engines accumulate into separate tiles (no shared-word writes).
- c[b] = xbar2 @ W2.T as two M=1 matmul passes sharing one PSUM tile
  (12 matmuls each), inserted after m-tile 12. Both results land on
  partition 0 where partition_broadcast sources them directly. Batch
  1 (needed first, by t16's fused evac) goes first.
- m-tiles 0-15 (batch 0) evacuate PSUM as plain copies; t16+ (batch 1)
  evacuate with the bias add fused on the DVE. The 16 deferred adds
  split: store units s0/s1 on the idle GpSimd engine, s2/s3 trickled
  1-per-slot on the DVE, so the DVE never head-blocks the PSUM
  evacuations; stores spread from ~50us on instead of bunching into
  the tail.
- Output stores: 786KB per 4-m-tile unit, alternating rings; the final
  unit splits 3+1 so the tail after the last matmul is one small DMA.

Host-side prep (layout/dtype only): partition-major SBUF images, bf16
casts (rel err ~3e-3 vs the 2e-2 gate), output stored bf16 and upcast
on the host. x2/W2 feed only the mean term (magnitude ~5% of output,
averaged over 2048 rows), so fp8 halves their HBM traffic at ~1e-3
output error.
"""

import sys

import numpy as np

# concourse normally comes from the axon site overlay already on sys.path;
# append /opt/trn_rl_repo as a fallback only.
if "/opt/trn_rl_repo" not in sys.path:
    sys.path.append("/opt/trn_rl_repo")

N_CORES = 8
B_PER_CORE = 2
L = 2048
D = 768  # d1 == d2 == d3 == 768
P = 128
NCH = D // P  # 6 contraction chunks
M = B_PER_CORE * L  # 4096 rows per core
NQ = 16  # x1 quarter groups, 2 m-tiles each
SPT = 4  # m-tiles per output store unit
NST = (M // P) // SPT  # 8 store units
K_DEFER = 16  # m-tiles evacuated before the bias c is ready (= batch 0)
N_VRED = 5  # x2 chunks reduced on Vector (the rest on Scalar)
C_AT = 16  # c-block emitted before this m-tile


def build_nc(debug=False):
    import concourse.bacc as bacc
    import concourse.mybir as mybir
    import concourse.tile as tile

    f32 = mybir.dt.float32
    bf16 = mybir.dt.bfloat16
    fp8 = mybir.dt.float8e4
    add = mybir.AluOpType.add
    Copy = mybir.ActivationFunctionType.Copy
    AxX = mybir.AxisListType.X

    nc = bacc.Bacc(None, target_bir_lowering=False, debug=debug)

    x1h = nc.declare_dram_parameter("x1h", [NQ, P, NCH, 2 * P], bf16, isOutput=False)
    x2h = nc.declare_dram_parameter("x2h", [B_PER_CORE, P, NCH, L], fp8, isOutput=False)
    w1h = nc.declare_dram_parameter("w1h", [P, NCH, D], bf16, isOutput=False)
    w2h = nc.declare_dram_parameter("w2h", [P, NCH, D], fp8, isOutput=False)
    bsh = nc.declare_dram_parameter("bsh", [B_PER_CORE, D], f32, isOutput=False)
    outh = nc.declare_dram_parameter("outh", [NST, P, SPT, D], bf16, isOutput=True)

    with tile.TileContext(nc) as tc:
        with (
            tc.tile_pool(name="const", bufs=1) as const,
            tc.tile_pool(name="x1p", bufs=1) as x1p,
            tc.tile_pool(name="x2p", bufs=1) as x2p,
            tc.tile_pool(name="yp", bufs=1) as yp,
            tc.tile_pool(name="psY", bufs=3, space="PSUM") as psY,
            tc.tile_pool(name="psC", bufs=1, space="PSUM") as psC,
        ):
            warm = const.tile([P, 512], bf16)
            nc.vector.memset(warm[:], 0.03125)

            # ---- DMA kickoff ----
            # sync ring: x1 q0 halves lead, then W1 odd chunks, rest of x1,
            # W2 slotted in mid-stream
            w1sb = const.tile([P, NCH, D], bf16)
            x1tiles = [
                x1p.tile([P, NCH, 2 * P], bf16, name=f"x1q{q}", tag=f"x1q{q}")
                for q in range(NQ)
            ]
            nc.sync.dma_start(x1tiles[0][:, 0:NCH:2, 0:P], x1h[0][:, 0:NCH:2, 0:P])
            q0adma = nc.sync.dma_start(
                x1tiles[0][:, 1:NCH:2, 0:P], x1h[0][:, 1:NCH:2, 0:P]
            )
            for c in (1, 3, 5):
                nc.sync.dma_start(w1sb[:, c : c + 1, :], w1h[:, c : c + 1, :])
            nc.sync.dma_start(
                x1tiles[0][:, :, P : 2 * P], x1h[0][:, :, P : 2 * P]
            )
            for q in (1, 2, 3, 4):
                nc.sync.dma_start(x1tiles[q][:], x1h[q])
            w2sb = const.tile([P, NCH, D], fp8)
            nc.sync.dma_start(w2sb[:], w2h[:])
            for q in range(5, NQ):
                nc.sync.dma_start(x1tiles[q][:], x1h[q])

            # scalar ring: W1 even chunks + bias row
            for c in (0, 2, 4):
                nc.scalar.dma_start(w1sb[:, c : c + 1, :], w1h[:, c : c + 1, :])
            bsum_sb = const.tile([B_PER_CORE, D], f32)
            nc.scalar.dma_start(bsum_sb[:], bsh[:])

            x2tiles = [
                x2p.tile([P, NCH, L], fp8, name=f"x2t{b}", tag=f"x2t{b}")
                for b in range(B_PER_CORE)
            ]

            # ---- warm-up matmuls: flip the HAM clock gate during DMA fill ----
            pc = psC.tile([1, D], f32)
            for _ in range(4):
                nc.tensor.matmul(
                    pc[:, 0:512], warm[:, 0:1], warm[:], start=True, stop=True
                )

            # ---- x2 mean accumulators (separate tiles per engine) ----
            xbts = const.tile([P, B_PER_CORE, NCH], f32)  # scalar-written
            xbtv = const.tile([P, N_VRED], f32)  # vector-written
            scr = const.tile([P, L], bf16)

            # all 12 x2 chunks ride the scalar ring, gated on x1 q0a via
            # the ring's FIFO (first DMA carries the sem dep). Chunk order:
            # b1 c0..c4 (Vector-reduced, needed first), then b1 c5 + b0
            # c0..c5 (Scalar-reduced) interleaved with their reductions so
            # the reduce chain paces the later transfers.
            v_chunks = [(1, c) for c in range(N_VRED)]
            s_chunks = [(1, c) for c in range(N_VRED, NCH)] + [
                (0, c) for c in range(NCH)
            ]

            def x2_sdma(b, c, gate=False):
                dma = nc.scalar.dma_start(x2tiles[b][:, c, :], x2h[b, :, c, :])
                if gate:
                    tile.add_dep_helper(
                        dma.ins, q0adma.ins, sync=True, reason="x2 yields to x1 q0"
                    )

            for j, (b, c) in enumerate(v_chunks):
                x2_sdma(b, c, gate=(j == 0))
            for j in range(3):
                x2_sdma(*s_chunks[j])
            for j, (b, c) in enumerate(s_chunks):
                nc.scalar.activation(
                    scr[:], x2tiles[b][:, c, :], Copy,
                    accum_out=xbts[:, b, c : c + 1],
                )
                if j + 3 < len(s_chunks):
                    x2_sdma(*s_chunks[j + 3])

            def emit_vector_reduce(v):
                nc.vector.tensor_reduce(
                    xbtv[:, v : v + 1], x2tiles[1][:, v, :], AxX, add
                )

            vec_red_at = {2: 0, 4: 1, 6: 2, 8: 3, 10: 4}

            # ---- main matmul stream ----
            cr = [None, None]
            ysu = [
                yp.tile([P, SPT, D], bf16, name=f"ys{s}", tag=f"ys{s}")
                for s in range(NST)
            ]

            def emit_mtile(t):
                q, sub = t // 2, t % 2
                xq = x1tiles[q]
                py_ = psY.tile([P, D], f32)
                for c in range(NCH):
                    xw = xq[:, c, sub * P : (sub + 1) * P]
                    nc.tensor.matmul(
                        py_[:, 0:512], xw, w1sb[:, c, 0:512],
                        start=(c == 0), stop=(c == NCH - 1),
                    )
                    nc.tensor.matmul(
                        py_[:, 512:768], xw, w1sb[:, c, 512:768],
                        start=(c == 0), stop=(c == NCH - 1),
                    )
                    if t == 0 and c < 3:
                        # junk matmuls fill m-tile 0's DMA-wait bubbles so
                        # the HAM activity window stays busy and the clock
                        # un-throttles before the stream locks in
                        nc.tensor.matmul(
                            pc[:, 0:512], warm[:, 0:1], warm[:],
                            start=True, stop=True,
                        )
                s, tl = t // SPT, t % SPT
                if t < K_DEFER:
                    nc.vector.tensor_copy(ysu[s][:, tl, :], py_[:])
                else:
                    nc.vector.tensor_tensor(
                        ysu[s][:, tl, :], py_[:], cr[t // 16][:], op=add
                    )

            def emit_c_block():
                # c[b] = (mean(x2[b]) @ W2.T + b1 + b2), two M=1 passes
                # sharing one PSUM tile so both rows land on partition 0.
                # Batch 1 first: its broadcast feeds t16's fused evac.
                xbtb = const.tile([P, B_PER_CORE, NCH], bf16)
                nc.gpsimd.tensor_scalar_mul(
                    xbtb[:, 0, :], xbts[:, 0, :], 1.0 / L
                )
                nc.gpsimd.tensor_scalar_mul(
                    xbtb[:, 1, N_VRED:NCH], xbts[:, 1, N_VRED:NCH], 1.0 / L
                )
                nc.gpsimd.tensor_scalar_mul(
                    xbtb[:, 1, 0:N_VRED], xbtv[:], 1.0 / L
                )
                for b in (1, 0):
                    for c in range(NCH):
                        nc.tensor.matmul(
                            pc[:, 0:512], xbtb[:, b, c : c + 1], w2sb[:, c, 0:512],
                            start=(c == 0), stop=(c == NCH - 1),
                        )
                    for c in range(NCH):
                        nc.tensor.matmul(
                            pc[:, 512:768], xbtb[:, b, c : c + 1],
                            w2sb[:, c, 512:768],
                            start=(c == 0), stop=(c == NCH - 1),
                        )
                    csb = const.tile([1, D], bf16, name=f"cs{b}", tag=f"cs{b}")
                    nc.vector.tensor_tensor(
                        csb[:], pc[:], bsum_sb[0:1, :], op=add
                    )
                    crb = const.tile([P, D], bf16, name=f"cr{b}", tag=f"cr{b}")
                    nc.gpsimd.partition_broadcast(crb[:], csb[:])
                    cr[b] = crb

            def emit_store(s, ring, lo=0, hi=SPT):
                ring.dma_start(outh[s][:, lo:hi, :], ysu[s][:, lo:hi, :])

            store_ring = [nc.sync, nc.scalar]
            n_store = 0
            dve_flush = [(s, tl) for s in (2, 3) for tl in range(SPT)]
            fi = 0
            for t in range(32):
                if t == C_AT:
                    emit_c_block()
                emit_mtile(t)
                if t in vec_red_at:
                    emit_vector_reduce(vec_red_at[t])
                if t == C_AT + 1:
                    # deferred adds for units s0/s1 on the idle GpSimd
                    # engine (queue reaches them right after cr0's bcast)
                    for s in (0, 1):
                        for tl in range(SPT):
                            nc.gpsimd.tensor_tensor(
                                ysu[s][:, tl, :], ysu[s][:, tl, :], cr[0][:],
                                op=add,
                            )
                        emit_store(s, store_ring[n_store % 2])
                        n_store += 1
                if t >= C_AT + 4 and fi < len(dve_flush):
                    # units s2/s3 trickle 1-per-slot on the DVE
                    s, tl = dve_flush[fi]
                    nc.vector.tensor_tensor(
                        ysu[s][:, tl, :], ysu[s][:, tl, :], cr[0][:], op=add
                    )
                    fi += 1
                    if tl == SPT - 1:
                        emit_store(s, store_ring[n_store % 2])
                        n_store += 1
                if t >= K_DEFER and t % SPT == SPT - 1 and t < 31:
                    emit_store(t // SPT, store_ring[n_store % 2])
                    n_store += 1
                if t == 30:
                    emit_store(7, store_ring[n_store % 2], 0, 3)
                    n_store += 1
                if t == 31:
                    emit_store(7, store_ring[n_store % 2], 3, 4)
                    n_store += 1
            while fi < len(dve_flush):
                s, tl = dve_flush[fi]
                nc.vector.tensor_tensor(
                    ysu[s][:, tl, :], ysu[s][:, tl, :], cr[0][:], op=add
                )
                fi += 1
                if tl == SPT - 1:
                    emit_store(s, store_ring[n_store % 2])
                    n_store += 1

    return nc


def make_in_maps(x1, x2, W1, b1, W2, b2):
    import ml_dtypes

    bf16 = ml_dtypes.bfloat16
    fp8 = ml_dtypes.float8_e4m3fn

    def wlayout(W, dt):
        # [e, d] -> W.T [d, e] -> [p, c, e] with d = c*128 + p
        wt = np.ascontiguousarray(W.T).reshape(NCH, P, D).transpose(1, 0, 2)
        return np.ascontiguousarray(wt).astype(dt)

    w1h = wlayout(W1, bf16)
    w2h = wlayout(W2, fp8)
    bsh = np.ascontiguousarray(
        np.broadcast_to((b1 + b2).reshape(1, D), (B_PER_CORE, D))
    ).astype(np.float32)
    in_maps = []
    for k in range(N_CORES):
        x1_s = x1[k * B_PER_CORE : (k + 1) * B_PER_CORE]  # [2, 2048, 768]
        x2_s = x2[k * B_PER_CORE : (k + 1) * B_PER_CORE]
        # x1t [d, m] with col m = b*2048 + i, then quarter-major image
        x1t = np.transpose(x1_s, (2, 0, 1)).reshape(D, M)
        x1h = np.ascontiguousarray(
            x1t.reshape(NCH, P, NQ, 2 * P).transpose(2, 1, 0, 3)
        ).astype(bf16)  # [q, p, c, m_in_quarter]
        # x2 transposed: [b, p, c, j] with d = c*128 + p
        x2h = np.ascontiguousarray(
            np.transpose(x2_s, (0, 2, 1)).reshape(B_PER_CORE, NCH, P, L)
            .transpose(0, 2, 1, 3)
        ).astype(fp8)
        in_maps.append(
            {"x1h": x1h, "x2h": x2h, "w1h": w1h, "w2h": w2h, "bsh": bsh}
        )
    return in_maps


def kernel(x1, x2, W1, b1, W2, b2, trace=False):
    from concourse.bass_utils import run_bass_kernel_spmd

    # accept jax arrays / lists transparently
    x1, x2, W1, b1, W2, b2 = (
        np.asarray(t, dtype=np.float32) for t in (x1, x2, W1, b1, W2, b2)
    )
    nc = build_nc(debug=False)
    nc.finalize()
    in_maps = make_in_maps(x1, x2, W1, b1, W2, b2)
    res = run_bass_kernel_spmd(
        nc, in_maps, core_ids=list(range(N_CORES)), trace=trace
    )
    shards = []
    for k in range(N_CORES):
        oh = res.results[k]["outh"]  # [NST, P, SPT, D] bf16, row = (s*SPT+t)*128+p
        flat = (
            oh.astype(np.float32).transpose(0, 2, 1, 3).reshape(M, D)
        )
        shards.append(flat.reshape(B_PER_CORE, L, D))
    out = np.concatenate(shards, axis=0)
    if trace:
        kernel.last_result = res
    return out
